# revision 16
# baseline (speedup 1.0000x reference)
"""Multi-head attention (B=2, N=2048, D=768, H=12) on 8 Trainium2 NeuronCores.

v4: collective-free (batch x head-group x query-half) sharding.

Core c = (b, hg, qh): batch b = c>>2, head-group hg = (c>>1)&1 (heads
hg*6..hg*6+5), query-half qh = c&1 (rows qh*1024..+1024). Each core
projects Q for its 1024 queries and K/V for its 6 heads over ALL 2048
keys locally -- the small redundant K/V projection replaces v3's 94us
AllGather -- then runs attention for 3 head-pairs x 2 query-tiles with
16-deep PSUM accumulation of AV (ones column gives softmax
denominators), and emits a PARTIAL output projection over its 384
head-dims. The host sums the two head-group partials per (b, qh) block
and adds b_proj (that is the unsharding step): no device collective.

The host packs x^T token-block-major with the core's OWN query blocks
at kb slots 0-1 and the remaining half at slots 2-3; key order is
permutation-invariant under softmax so K/V just use that order, and the
single SPMD program never needs to branch on the core id.

Schedule: pair j+1 K/Q projections and the V blocks are interleaved
into the running pair's chunk loop so ACT never starves; scores use
row-tiled concurrent 64-partition matmul pairs; exp reads 1024-wide f32
PSUM and writes bf16 SBUF.
"""

import sys

sys.path.insert(0, "/opt/trn_rl_repo")

import numpy as np

import concourse.bass as bass
import concourse.mybir as mybir
import concourse.tile as tile
from concourse import bacc
from concourse import bass_utils

B, N, D = 2, 2048, 768
H, DH = 12, 64
NCORES = 8
NPAIR = 3          # head pairs per core
KC = D // 128      # contraction chunks (6)
NKB = 4            # 512-token key blocks
NM = 16            # 128-token key chunks
NQT = 2            # 512-query tiles per core
SCALE = DH ** -0.5

f32 = mybir.dt.float32
bf16 = mybir.dt.bfloat16
EXP = mybir.ActivationFunctionType.Exp

_CACHE = {}


def _build():
    nc = bacc.Bacc("TRN2", target_bir_lowering=False, debug=False,
                   enable_asserts=False, num_devices=NCORES)
    # x^T token-block-major: [128, kb(4) * c(6) * 512]
    xh = nc.dram_tensor("xh", [128, NKB * KC * 512], bf16,
                        kind="ExternalInput").ap()
    # Wq/Wk pair-major: block (j, c) at cols (j*KC + c)*128
    whq = nc.dram_tensor("whq", [128, NPAIR * KC * 128], bf16,
                         kind="ExternalInput").ap()
    whk = nc.dram_tensor("whk", [128, NPAIR * KC * 128], bf16,
                         kind="ExternalInput").ap()
    # Wv chunk-major: [128, c(6) * 384]
    whv = nc.dram_tensor("whv", [128, KC * 384], bf16,
                         kind="ExternalInput").ap()
    # W_proj rows for this head-group, pair-major: [128, j(3) * 768]
    whp = nc.dram_tensor("whp", [128, NPAIR * 768], bf16,
                         kind="ExternalInput").ap()
    # col j: Q bias pair j; 3+j: K bias; 6+j: V bias
    bqkv = nc.dram_tensor("bqkv", [128, 9], f32, kind="ExternalInput").ap()
    out = nc.dram_tensor("out", [1024, D], f32, kind="ExternalOutput").ap()

    with tile.TileContext(nc) as tc:
        from contextlib import ExitStack
        with ExitStack() as stack:
            ep = lambda *a, **k: stack.enter_context(tc.tile_pool(*a, **k))
            consts = ep(name="consts", bufs=1)
            w_pool = ep(name="w_pool", bufs=1)
            k_pool = ep(name="k_pool", bufs=1)
            q_pool = ep(name="q_pool", bufs=1)
            v_pool = ep(name="v_pool", bufs=1)
            pt_pool = ep(name="pt_pool", bufs=8)
            at_pool = ep(name="at_pool", bufs=1)
            nrm_pool = ep(name="nrm_pool", bufs=2)
            outp = ep(name="outp", bufs=2)
            ps_sc = ep(name="ps_sc", bufs=2, space="PSUM")
            ps_av = ep(name="ps_av", bufs=2, space="PSUM")

            # ---- inputs: first-needed pieces on parallel queues ----
            xh_sb = w_pool.tile([128, NKB * KC * 512], bf16, name="xh_sb")
            bq_sb = consts.tile([128, 9], f32)
            whk0 = w_pool.tile([128, KC * 128], bf16, name="whk0")
            whq0 = w_pool.tile([128, KC * 128], bf16, name="whq0")
            whv_sb = w_pool.tile([128, KC * 384], bf16, name="whv_sb")
            nc.sync.dma_start(out=xh_sb[:, 0:1536], in_=xh[:, 0:1536])
            nc.scalar.dma_start(out=xh_sb[:, 1536:3072],
                                in_=xh[:, 1536:3072])
            nc.gpsimd.dma_start(out=bq_sb, in_=bqkv)
            nc.gpsimd.dma_start(out=whk0, in_=whk[:, 0:KC * 128])
            nc.gpsimd.dma_start(out=whq0, in_=whq[:, 0:KC * 128])
            nc.scalar.dma_start(out=whv_sb, in_=whv)
            nc.sync.dma_start(out=xh_sb[:, 3072:2 * 3072],
                              in_=xh[:, 3072:2 * 3072])
            nc.scalar.dma_start(out=xh_sb[:, 2 * 3072:3 * 3072],
                                in_=xh[:, 2 * 3072:3 * 3072])
            nc.sync.dma_start(out=xh_sb[:, 3 * 3072:4 * 3072],
                              in_=xh[:, 3 * 3072:4 * 3072])
            whkr = w_pool.tile([128, 2 * KC * 128], bf16, name="whkr")
            nc.sync.dma_start(out=whkr, in_=whk[:, KC * 128:])
            whqr = w_pool.tile([128, 2 * KC * 128], bf16, name="whqr")
            nc.scalar.dma_start(out=whqr, in_=whq[:, KC * 128:])
            # PE warmup: dummy matmuls on scratch ramp HAM to 2.4GHz while
            # the input DMAs land (results never read)
            scr = consts.tile([128, 512], bf16, name="scr")
            nc.vector.memset(scr, 0.0)
            dmy = ps_sc.tile([128, 512], f32, name="dmy", tag="fill", bufs=1)
            for i in range(22):
                nc.tensor.matmul(dmy, scr[:, 0:128], scr,
                                 start=True, stop=True)
            # warmup exp to preload the ACT table off the critical path
            warm = consts.tile([1, 8], f32)
            nc.vector.memset(warm, 0.0)
            nc.scalar.activation(warm, warm, EXP)
            whp_sb = w_pool.tile([128, NPAIR * 768], bf16, name="whp_sb")
            nc.gpsimd.dma_start(out=whp_sb, in_=whp)

            def wq_jc(j, c):
                if j == 0:
                    return whq0[:, c * 128:(c + 1) * 128]
                base = ((j - 1) * KC + c) * 128
                return whqr[:, base:base + 128]

            def wk_jc(j, c):
                if j == 0:
                    return whk0[:, c * 128:(c + 1) * 128]
                base = ((j - 1) * KC + c) * 128
                return whkr[:, base:base + 128]

            def xt(c, kb):
                return xh_sb[:, kb * 3072 + c * 512: kb * 3072 + (c + 1) * 512]

            # ---- persistent tiles ----
            kown = k_pool.tile([128, NPAIR * 2048], bf16, name="kown")
            qsl = q_pool.tile([128, NPAIR * 1024], bf16, name="qsl")
            vsh = v_pool.tile([128, NM * NPAIR * 130 + 63], bf16, name="vsh")
            at = [at_pool.tile([128, 1024], bf16, name=f"at{j}", tag=f"at{j}")
                  for j in range(NPAIR)]
            vr = vsh[:, 0:NM * NPAIR * 130].rearrange(
                "p (m j t h) -> p m j t h", m=NM, j=NPAIR, t=2)
            nc.vector.memset(vsh[:, NM * NPAIR * 130:], 0.0)
            nc.vector.memset(vr[:, :, :, :, 64:65], 1.0)

            # ---- emission helpers ----
            def emit_k(j, kb):
                kp = ps_sc.tile([128, 512], f32, name=f"kp{j}_{kb}", tag="fill", bufs=1)
                for c in range(KC):
                    nc.tensor.matmul(kp, wk_jc(j, c), xt(c, kb),
                                     start=(c == 0), stop=(c == KC - 1))
                nc.vector.tensor_scalar_add(
                    kown[:, j * 2048 + kb * 512: j * 2048 + (kb + 1) * 512],
                    kp, bq_sb[:, 3 + j:4 + j])

            def emit_q(j, qt):
                # query half's token blocks are packed at kb slots 0 and 1
                qp = ps_sc.tile([128, 512], f32, name=f"qp{j}_{qt}", tag="fill", bufs=1)
                for c in range(KC):
                    nc.tensor.matmul(qp, wq_jc(j, c), xt(c, qt),
                                     start=(c == 0), stop=(c == KC - 1))
                nc.vector.tensor_scalar_add(
                    qsl[:, j * 1024 + qt * 512: j * 1024 + (qt + 1) * 512],
                    qp, bq_sb[:, j:j + 1])

            def emit_v(m):
                vp = ps_sc.tile([128, 384], f32, name=f"vp{m}", tag="fill", bufs=1)
                base = (m // 4) * 3072 + (m % 4) * 128
                for c in range(KC):
                    nc.tensor.matmul(
                        vp, xh_sb[:, base + c * 512: base + c * 512 + 128],
                        whv_sb[:, c * 384:(c + 1) * 384],
                        start=(c == 0), stop=(c == KC - 1))
                nc.vector.tensor_copy(
                    vr[:, m, :, :, 0:64],
                    vp.rearrange("p (j t h) -> p j t h", j=NPAIR, t=2))

            def vslab(m, j, t):
                o = (m * NPAIR + j) * 130 + t * 65
                return vsh[:, o:o + 65]

            def emit_sc(j, qt, m):
                # scores for key chunk m (row-tiled concurrent pair) + exp
                kc_ = kown[:, j * 2048 + m * 128: j * 2048 + (m + 1) * 128]
                q2 = qsl[:, j * 1024 + qt * 512: j * 1024 + (qt + 1) * 512]
                sc = ps_sc.tile([128, 1024], f32, name=f"sc{j}_{qt}_{m}",
                                tag="sc")
                nc.tensor.matmul(sc[:, 0:512], kc_[0:64, :], q2[0:64, :],
                                 start=True, stop=True)
                nc.tensor.matmul(sc[:, 512:1024], kc_[64:128, :],
                                 q2[64:128, :], start=True, stop=True)
                pt = pt_pool.tile([128, 1024], bf16, name=f"p{j}_{qt}_{m}",
                                  tag="pt")
                nc.scalar.activation(pt, sc, EXP, scale=SCALE)
                return pt

            def emit_av(j, m, pt, av_e, av_o):
                nc.tensor.matmul(av_e, vslab(m, j, 0), pt[:, 0:512],
                                 start=(m == 0), stop=(m == NM - 1))
                nc.tensor.matmul(av_o, vslab(m, j, 1), pt[:, 512:1024],
                                 start=(m == 0), stop=(m == NM - 1))

            def emit_normalize(j, qt, av_e, av_o):
                sums2 = nrm_pool.tile([1, 1024], f32, name=f"sums{j}_{qt}",
                                      tag="sums")
                nc.vector.tensor_copy(sums2[0:1, 0:512], av_e[64:65, :])
                nc.vector.tensor_copy(sums2[0:1, 512:1024], av_o[64:65, :])
                rec = nrm_pool.tile([1, 1024], f32, name=f"rec{j}_{qt}",
                                    tag="rec")
                nc.vector.reciprocal_approx_fast(rec, sums2)
                bc_e = nrm_pool.tile([64, 512], f32, name=f"bce{j}_{qt}",
                                     tag="bce")
                nc.gpsimd.partition_broadcast(bc_e, rec[0:1, 0:512])
                bc_o = nrm_pool.tile([64, 512], f32, name=f"bco{j}_{qt}",
                                     tag="bco")
                nc.gpsimd.partition_broadcast(bc_o, rec[0:1, 512:1024])
                a_e = at[j][0:64, qt * 512:(qt + 1) * 512]
                a_o = at[j][64:128, qt * 512:(qt + 1) * 512]
                nc.vector.tensor_mul(a_e, av_e[0:64, :], bc_e)
                nc.vector.tensor_mul(a_o, av_o[0:64, :], bc_o)
                nc.vector.tensor_scalar_add(a_e, a_e, bq_sb[0:64, 6 + j:7 + j])
                nc.vector.tensor_scalar_add(a_o, a_o,
                                            bq_sb[64:128, 6 + j:7 + j])

            def proj_closures(qt, tags):
                # partial out projection, one closure per 384-col half of a
                # 128-query tile (no bias: host adds b_proj after summing
                # head-group partials)
                cls = []
                for mt in range(4):
                    q0 = qt * 512 + mt * 128
                    st = {}
                    def half(h, q0=q0, mt=mt, st=st):
                        if h == 0:
                            st["ot"] = outp.tile([128, D], f32,
                                                 name=f"ot{qt}_{mt}",
                                                 tag="ot")
                        pj = ps_sc.tile([128, 384], f32,
                                        name=f"pj{qt}_{mt}_{h}",
                                        tag=tags[h], bufs=1)
                        for j in range(NPAIR):
                            nc.tensor.matmul(
                                pj, at[j][:, q0:q0 + 128],
                                whp_sb[:, j * 768 + h * 384:
                                       j * 768 + (h + 1) * 384],
                                start=(j == 0), stop=(j == NPAIR - 1))
                        nc.vector.tensor_copy(
                            st["ot"][:, h * 384:(h + 1) * 384], pj)
                        if h == 1:
                            eng = (nc.sync, nc.scalar, nc.gpsimd)[mt % 3]
                            eng.dma_start(out=out[q0:q0 + 128, :],
                                          in_=st["ot"])
                    cls.append(lambda h=0, f=half: f(0))
                    cls.append(lambda h=1, f=half: f(1))
                return cls

            # ---- main pipeline ----
            # micro-fillers: one PE matmul per closure, spread across the
            # chunk loops so ACT never starves behind a 6-matmul block
            def k_fillers(j, kb):
                st = {}
                def mm(c):
                    if c == 0:
                        st["p"] = ps_sc.tile([128, 512], f32,
                                             name=f"kp{j}_{kb}", tag="fill",
                                             bufs=1)
                    nc.tensor.matmul(st["p"], wk_jc(j, c), xt(c, kb),
                                     start=(c == 0), stop=(c == KC - 1))
                    if c == KC - 1:
                        nc.vector.tensor_scalar_add(
                            kown[:, j * 2048 + kb * 512:
                                 j * 2048 + (kb + 1) * 512],
                            st["p"], bq_sb[:, 3 + j:4 + j])
                return [lambda c=c: mm(c) for c in range(KC)]

            def q_fillers(j, qt):
                st = {}
                def mm(c):
                    if c == 0:
                        st["p"] = ps_sc.tile([128, 512], f32,
                                             name=f"qp{j}_{qt}", tag="fill",
                                             bufs=1)
                    nc.tensor.matmul(st["p"], wq_jc(j, c), xt(c, qt),
                                     start=(c == 0), stop=(c == KC - 1))
                    if c == KC - 1:
                        nc.vector.tensor_scalar_add(
                            qsl[:, j * 1024 + qt * 512:
                                j * 1024 + (qt + 1) * 512],
                            st["p"], bq_sb[:, j:j + 1])
                return [lambda c=c: mm(c) for c in range(KC)]

            # pair-0 head start: narrow K (keys 0-127 only) and
            # Q(0,0) before the loop so the first exp issues ~15us earlier;
            # the rest of key-block 0 and V(0) follow the first score.
            kpn = ps_sc.tile([128, 128], f32, name="kpn", tag="fill", bufs=1)
            for c in range(KC):
                nc.tensor.matmul(kpn, wk_jc(0, c), xt(c, 0)[:, 0:128],
                                 start=(c == 0), stop=(c == KC - 1))
            nc.vector.tensor_scalar_add(kown[:, 0:128], kpn,
                                        bq_sb[:, 3:4])
            for f in q_fillers(0, 0):
                f()

            def k_rest_fillers():
                st = {}
                def mm(c):
                    if c == 0:
                        st["p"] = ps_sc.tile([128, 384], f32, name="kpr",
                                             tag="fill", bufs=1)
                    nc.tensor.matmul(st["p"], wk_jc(0, c),
                                     xt(c, 0)[:, 128:512],
                                     start=(c == 0), stop=(c == KC - 1))
                    if c == KC - 1:
                        nc.vector.tensor_scalar_add(
                            kown[:, 128:512], st["p"], bq_sb[:, 3:4])
                return [lambda c=c: mm(c) for c in range(KC)]

            # (j, qt) -> [fillers...][chunk_slot]; V(m) stays just-in-time
            # inside (0,0) (emitted immediately before chunk m), the rest
            # spreads round-robin
            pre = {}
            pre[(0, 0)] = {0: k_fillers(0, 1), 1: k_fillers(0, 2),
                           2: k_fillers(0, 3), 3: q_fillers(0, 1)}
            rr = {
                (0, 1): k_fillers(1, 0) + k_fillers(1, 1) + q_fillers(1, 0),
                (1, 0): k_fillers(1, 2) + k_fillers(1, 3) + q_fillers(1, 1),
                (1, 1): k_fillers(2, 0) + k_fillers(2, 1) + q_fillers(2, 0),
                (2, 0): k_fillers(2, 2) + k_fillers(2, 3) + q_fillers(2, 1),
            }

            for j in range(NPAIR):
                for qt in range(NQT):
                    grp = pre.get((j, qt), {})
                    flat = rr.get((j, qt), [])
                    if (j, qt) == (2, 1):
                        # overlap qt0's projection with the last chunk loop
                        flat = flat + proj_closures(0, ("pj", "pj"))
                    av_e = ps_av.tile([65, 512], f32, name=f"ave{j}_{qt}",
                                      tag="av")
                    av_o = ps_av.tile([65, 512], f32, name=f"avo{j}_{qt}",
                                      tag="av")
                    for m in range(NM):
                        if (j, qt, m) == (0, 0, 0):
                            pt = emit_sc(0, 0, 0)
                            emit_v(0)
                            emit_av(0, 0, pt, av_e, av_o)
                            for f in k_rest_fillers():
                                f()
                            for f in grp.get(m, []):
                                f()
                            continue
                        if j == 0 and qt == 0:
                            emit_v(m)
                        lo = m * len(flat) // NM
                        hi = (m + 1) * len(flat) // NM
                        for f in flat[lo:hi]:
                            f()
                        pt = emit_sc(j, qt, m)
                        emit_av(j, m, pt, av_e, av_o)
                        for f in grp.get(m, []):
                            f()
                    emit_normalize(j, qt, av_e, av_o)
            # tail: qt1 projection on the now-idle sc ring (double-buffered)
            for f in proj_closures(1, ("fill", "pj")):
                f()

    nc.compile()
    return nc


def get_nc():
    if "nc" not in _CACHE:
        _CACHE["nc"] = _build()
    return _CACHE["nc"]


def make_in_maps(x, W_qkv, b_qkv, W_proj, b_proj):
    import ml_dtypes
    bf = ml_dtypes.bfloat16
    x = np.asarray(x, dtype=np.float32)
    W_qkv = np.asarray(W_qkv, dtype=np.float32)
    b_qkv = np.asarray(b_qkv, dtype=np.float32)
    W_proj = np.asarray(W_proj, dtype=np.float32)

    cp = np.ascontiguousarray
    per_hg = {}
    for hg in range(2):
        s = hg * 384
        whq = cp(W_qkv[:, s:s + 384].astype(bf).reshape(KC, 128, NPAIR, 128)
                 .transpose(1, 2, 0, 3).reshape(128, NPAIR * KC * 128))
        whk = cp(W_qkv[:, 768 + s:768 + s + 384].astype(bf)
                 .reshape(KC, 128, NPAIR, 128)
                 .transpose(1, 2, 0, 3).reshape(128, NPAIR * KC * 128))
        whv = cp(W_qkv[:, 1536 + s:1536 + s + 384].astype(bf)
                 .reshape(KC, 128, 384).transpose(1, 0, 2)
                 .reshape(128, KC * 384))
        whp = cp(W_proj[s:s + 384, :].astype(bf).reshape(NPAIR, 128, 768)
                 .transpose(1, 0, 2).reshape(128, NPAIR * 768))
        cols = []
        for j in range(NPAIR):
            cols.append(b_qkv[s + j * 128: s + (j + 1) * 128])
        for j in range(NPAIR):
            cols.append(b_qkv[768 + s + j * 128: 768 + s + (j + 1) * 128])
        for j in range(NPAIR):
            cols.append(b_qkv[1536 + s + j * 128: 1536 + s + (j + 1) * 128])
        bq2d = cp(np.stack(cols, axis=1))
        per_hg[hg] = (whq, whk, whv, whp, bq2d)

    # x^T token-block-major, per (b, qh): own query blocks at kb 0-1
    xt_b = {}
    for b in range(B):
        xb = x[b].T.astype(bf).reshape(KC, 128, NKB, 512)  # [c,p,kb,t]
        for qh in range(2):
            order = [2 * qh, 2 * qh + 1, 2 * (1 - qh), 2 * (1 - qh) + 1]
            xt_b[(b, qh)] = cp(xb[:, :, order, :].transpose(1, 2, 0, 3)
                               .reshape(128, NKB * KC * 512))

    in_maps = []
    for c in range(NCORES):
        b, hg, qh = c >> 2, (c >> 1) & 1, c & 1
        whq, whk, whv, whp, bq2d = per_hg[hg]
        in_maps.append({"xh": xt_b[(b, qh)], "whq": whq, "whk": whk,
                        "whv": whv, "whp": whp, "bqkv": bq2d})
    return in_maps


def run(in_maps, **kw):
    return bass_utils.run_bass_kernel_spmd(get_nc(), in_maps,
                                           core_ids=list(range(NCORES)), **kw)


def assemble(results, b_proj):
    out = np.empty((B, N, D), dtype=np.float32)
    bp = np.asarray(b_proj, dtype=np.float32)
    for b in range(B):
        for qh in range(2):
            p0 = results[(b << 2) | (0 << 1) | qh]["out"]
            p1 = results[(b << 2) | (1 << 1) | qh]["out"]
            out[b, qh * 1024:(qh + 1) * 1024] = p0 + p1 + bp
    return out


def kernel(x, W_qkv, b_qkv, W_proj, b_proj):
    in_maps = make_in_maps(x, W_qkv, b_qkv, W_proj, b_proj)
    res = run(in_maps)
    return assemble(res.results, b_proj)
